# revision 3
# baseline (speedup 1.0000x reference)
"""Binarized 3x3 conv (BConv) Trainium2 Bass kernel — 1D row-Winograd F(2,3).

Problem: x[32,256,56,56] f32, W[256,256,3,3] f32.
  out = conv2d(x, sign(W), stride 1, pad 1)  (NCHW / OIHW)

Strategy:
  - Data-parallel over batch: 8 cores x 4 images each, identical SPMD program.
  - Winograd F(2,3) applied along the ROW axis only (columns stay direct):
    for an output row pair (2t, 2t+1) with padded input rows d_r = xpad[2t+r],
      V0 = d0-d2, V1 = d1+d2, V2 = d2-d1, V3 = d1-d3        (input transform)
      U0 = g0, U1 = (g0+g1+g2)/2, U2 = (g0-g1+g2)/2, U3 = g2 (g = sign(w) rows)
      M_i = V_i (x) U_i  (1x3 col-conv, contraction over C_in -> PE matmuls)
      out_even = M0+M1+M2, out_odd = M1-M2-M3               (output transform)
    24 matmuls per 7-pair group vs 36 direct -> PE time ~2/3 of direct conv.
  - Input transform: cast f32->bf16 padded image (ACT/GpSimd), then 4 DVE
    tensor_tensor ops per image-half produce V[4, 28 pairs, 58 padded cols].
  - Weights: sign via tensor_scalar is_ge trick (gives sign/2), U built with
    6 small DVE adds per input-half; all U values exact in bf16.
  - Output: ACT evacuates M_i PSUM->SBUF bf16, DVE combines (bf16 2x mode),
    bf16 DMA out; host view-upcasts to f32 (exact, no arithmetic).
"""

import sys
from contextlib import ExitStack
from itertools import product

sys.path.insert(0, "/opt/trn_rl_repo")

import numpy as np

import concourse.mybir as mybir
import concourse.tile as tile
from concourse import bacc
from concourse.bass_utils import run_bass_kernel_spmd

N_CORES = 8
NIMG = 4          # images per core (32 / 8)
C = 256           # channels (in == out)
H = 56
HP = H + 2        # padded spatial
P = 128           # partitions
T = H // 2        # 28 output-row pairs
PPG = 7           # pairs per PSUM group -> N = 7*56 = 392
NG = T // PPG     # 4 groups per (img, oc-half)
PIECES = [(0, 7), (7, 14), (14, 28)]  # V build pieces (pairs)

F32 = mybir.dt.float32
BF16 = mybir.dt.bfloat16

_cached = {}


def build_program():
    nc = bacc.Bacc("TRN2", target_bir_lowering=False, debug=False,
                   num_devices=N_CORES)

    x_d = nc.dram_tensor("x", [NIMG, C, H, H], F32, kind="ExternalInput")
    # W host-permuted (layout only) to [C_in, kh, kw, C_out] so weight-tap
    # slices are contiguous lhsT tiles
    w_d = nc.dram_tensor("W", [C, 3, 3, C], F32, kind="ExternalInput")
    y_d = nc.dram_tensor("y", [NIMG, C, H, H], BF16, kind="ExternalOutput")

    with tile.TileContext(nc) as tc, ExitStack() as ctx:
        wst_pool = ctx.enter_context(tc.tile_pool(name="wst", bufs=2))
        s_pool = ctx.enter_context(tc.tile_pool(name="sgn", bufs=2))
        u_pool = ctx.enter_context(tc.tile_pool(name="u", bufs=2))
        stage_pool = ctx.enter_context(tc.tile_pool(name="stage", bufs=4))
        v_pool = ctx.enter_context(tc.tile_pool(name="v", bufs=6))
        osb_pool = ctx.enter_context(tc.tile_pool(name="osb", bufs=6))
        ev_pool = ctx.enter_context(tc.tile_pool(name="ev", bufs=8))
        tmp_pool = ctx.enter_context(tc.tile_pool(name="tmp", bufs=4))
        psum_pool = ctx.enter_context(tc.tile_pool(name="ps", bufs=8,
                                                   space="PSUM"))

        # ---- weight prep: DMA -> sign/2 (bf16) -> U tiles [P, 4i, 3ct, 256]
        u_tiles = []
        s_tiles = []

        wst_tiles = []

        def prep_w_alloc(ic):
            wst = wst_pool.tile([P, 3, 3, C], F32, tag="wst",
                                name=f"wst_{ic}")
            wst_tiles.append(wst)
            return wst

        def prep_w_dma(ic, eng_big, eng_small):
            # split each W half 2/3 + 1/3 across both HWDGE rings: a single
            # ring moves only ~200GB/s and W gates the whole startup chain
            wst = wst_tiles[ic]
            eng_big.dma_start(wst[:, 0:2], w_d[ic * P:(ic + 1) * P, 0:2])
            eng_small.dma_start(wst[:, 2:3], w_d[ic * P:(ic + 1) * P, 2:3])

        def prep_s_alloc(ic):
            s = s_pool.tile([P, 3, 3, C], BF16, tag="sgn", name=f"s_{ic}")
            u = u_pool.tile([P, 4, 3, C], BF16, tag="u", name=f"u_{ic}")
            s_tiles.append(s)
            u_tiles.append(u)

        def prep_s(ic, oc):
            # s = (w>=0) - 0.5 = sign(w)/2  (exact in bf16), per oc-half
            o0, o1 = oc * P, (oc + 1) * P
            nc.vector.tensor_scalar(
                s_tiles[ic][:, :, :, o0:o1], wst_tiles[ic][:, :, :, o0:o1],
                0.0, 0.5,
                mybir.AluOpType.is_ge, mybir.AluOpType.subtract)

        def prep_u(oc, ic):
            o0, o1 = oc * P, (oc + 1) * P
            s, u = s_tiles[ic], u_tiles[ic]
            s0 = s[:, 0, :, o0:o1]
            s1 = s[:, 1, :, o0:o1]
            s2 = s[:, 2, :, o0:o1]
            # U0 = 2*s0 = sign(g0); U3 = 2*s2  (single-src op -> 4x mode)
            nc.vector.tensor_scalar_mul(u[:, 0, :, o0:o1], s0, 2.0)
            nc.vector.tensor_scalar_mul(u[:, 3, :, o0:o1], s2, 2.0)
            # U1 = s0+s1+s2 ; U2 = s0-s1+s2  (values in {±.5, ±1.5})
            nc.vector.tensor_add(u[:, 1, :, o0:o1], s0, s1)
            nc.vector.tensor_add(u[:, 1, :, o0:o1], u[:, 1, :, o0:o1], s2)
            nc.vector.tensor_sub(u[:, 2, :, o0:o1], s0, s1)
            nc.vector.tensor_add(u[:, 2, :, o0:o1], u[:, 2, :, o0:o1], s2)

        # ---- image load: DMA f32 into a zero-padded stage, then 4 DVE
        # tensor_tensor ops per piece build V straight from f32 (1x mode --
        # same throughput as cast+bf16-2x, but no cast op and no dep chain)
        def make_loader(img, ic):
            # stage pads ROWS only (cols stay 56) so input DMAs have fully
            # contiguous destinations -- a strided DMA dst costs ~5x in
            # descriptor generation on the issuing ring
            stage = stage_pool.tile([P, HP, H], F32, tag="stage",
                                    name=f"stage_{img}_{ic}")
            nc.gpsimd.memset(stage[:, 0, :], 0.0)
            nc.gpsimd.memset(stage[:, HP - 1, :], 0.0)
            v = v_pool.tile([P, 4, T, HP], BF16, tag="v",
                            name=f"v_{img}_{ic}")
            # V column borders are zero (padded input cols)
            nc.gpsimd.memset(v[:, :, :, 0], 0.0)
            nc.gpsimd.memset(v[:, :, :, HP - 1], 0.0)
            # inputs: ic0 on the sync HWDGE ring, ic1 on the ACT ring
            # (outputs go out via the GpSimd SWDGE ring)
            dma_eng = nc.sync if ic == 0 else nc.scalar

            def do_dma(pi):
                t0, t1 = PIECES[pi]
                r0 = 1 if pi == 0 else 2 * t0 + 2
                r1 = HP - 1 if pi == len(PIECES) - 1 else 2 * t1 + 2
                dma_eng.dma_start(stage[:, r0:r1, :],
                                  x_d[img, ic * P:(ic + 1) * P,
                                      r0 - 1:r1 - 1])

            def do_v(pi):
                t0, t1 = PIECES[pi]
                n = t1 - t0

                def row(off):
                    a = 2 * t0 + off
                    return stage[:, a:min(a + 2 * n, HP):2, :]

                nc.vector.tensor_sub(v[:, 0, t0:t1, 1:HP - 1],
                                     row(0), row(2))
                nc.vector.tensor_add(v[:, 1, t0:t1, 1:HP - 1],
                                     row(1), row(2))
                nc.vector.tensor_sub(v[:, 2, t0:t1, 1:HP - 1],
                                     row(2), row(1))
                nc.vector.tensor_sub(v[:, 3, t0:t1, 1:HP - 1],
                                     row(1), row(3))

            return do_dma, do_v, v

        def load_image(img):
            vs = []
            for ic in range(2):
                do_dma, do_v, v = make_loader(img, ic)
                for pi in range(len(PIECES)):
                    do_dma(pi)
                    do_v(pi)
                vs.append(v)
            return vs

        warm = []

        # ---- conv one (img, oc-half, group): 24 matmuls + evac/combine
        def conv_group(img, oc, g, vs, splits=None):
            o0, o1 = oc * P, (oc + 1) * P
            for p0, p1 in (splits or ((0, PPG),)):
                t0 = g * PPG + p0
                npr = p1 - p0
                psums = [psum_pool.tile([P, npr, H], F32, tag="ps",
                                        name=f"m_{img}_{oc}_{g}_{i}_{p0}")
                         for i in range(4)]
                order = [(i, s, ic, ct) for i in range(4)
                         for s, (ic, ct) in
                         enumerate(product(range(2), range(3)))]
                for i, step, ic, ct in order:
                    nc.tensor.matmul(
                        psums[i][:],
                        u_tiles[ic][:, i, ct, o0:o1],
                        vs[ic][:, i, t0:t0 + npr, ct:ct + H],
                        start=(step == 0),
                        stop=(step == 5),
                    )
                osb = osb_pool.tile([P, 2 * npr, H], BF16, tag="osb",
                                    name=f"osb_{img}_{oc}_{g}_{p0}")
                t01 = tmp_pool.tile([P, npr, H], BF16, tag="tmp",
                                    name=f"t01_{img}_{oc}_{g}_{p0}")
                t12 = tmp_pool.tile([P, npr, H], BF16, tag="tmp",
                                    name=f"t12_{img}_{oc}_{g}_{p0}")
                # ACT evacuates PSUM->SBUF bf16, DVE combines in 2x bf16
                e = [ev_pool.tile([P, npr, H], BF16, tag="ev",
                                  name=f"e_{img}_{oc}_{g}_{i}_{p0}")
                     for i in range(4)]
                for i in range(4):
                    nc.scalar.copy(e[i][:], psums[i][:])
                nc.vector.tensor_add(t01[:], e[0][:], e[1][:])
                nc.vector.tensor_add(osb[:, 0:2 * npr:2, :],
                                     t01[:], e[2][:])
                nc.vector.tensor_sub(t12[:], e[1][:], e[2][:])
                nc.vector.tensor_sub(osb[:, 1:2 * npr:2, :],
                                     t12[:], e[3][:])
                r0 = 2 * t0
                out_eng = nc.sync if (img == NIMG - 1 and oc == 1
                                      and g == NG - 1) else nc.gpsimd
                out_eng.dma_start(
                    y_d[img, o0:o1, r0:r0 + 2 * npr, :], osb[:])

        def conv_img(img, oc, vs, last=False):
            for g in range(NG):
                if last and g == NG - 1:
                    conv_group(img, oc, g, vs, splits=((0, 4), (4, PPG)))
                else:
                    conv_group(img, oc, g, vs)

        # ---- program order: startup interleaved so the DVE FIFO matches
        # matmul consumption order; piece 0 covers exactly group 0's pairs
        dma00, v00, vt00 = make_loader(0, 0)
        dma01, v01, vt01 = make_loader(0, 1)
        # PE warm-up: ~32 junk matmuls on a zeroed tile keep the PE busy
        # from ~7.5us so the HAM clock gate opens (1.2 -> 2.4 GHz) before
        # the first real matmul group issues
        warm_sb = tmp_pool.tile([P, P], BF16, tag="tmp", name="warm_sb")
        nc.gpsimd.memset(warm_sb[:], 0.0)
        warm.append(warm_sb)
        warm_ps = psum_pool.tile([P, PPG, H], F32, tag="ps", name="warm_ps")
        for _ in range(32):
            nc.tensor.matmul(warm_ps[:, 0, :], warm_sb[:],
                             warm_sb[:, 0:H], start=True, stop=True)
        prep_w_alloc(0)
        prep_w_alloc(1)
        prep_w_dma(0, nc.scalar, nc.sync)   # rings: [W0a|x01p0|W1b] (ACT)
        dma00(0)                            #        [W0b|x00p0|W1a] (sync)
        dma01(0)
        prep_w_dma(1, nc.sync, nc.scalar)
        prep_s_alloc(0)
        prep_s_alloc(1)
        prep_s(0, 0)
        prep_s(1, 0)
        prep_u(0, 0)
        prep_u(0, 1)
        v00(0)
        v01(0)
        v0 = [vt00, vt01]
        conv_group(0, 0, 0, v0)
        dma00(1)
        dma01(1)
        v00(1)
        v01(1)
        prep_s(0, 1)
        prep_s(1, 1)
        prep_u(1, 0)
        prep_u(1, 1)
        conv_group(0, 0, 1, v0)
        dma00(2)
        dma01(2)
        v00(2)
        v01(2)
        conv_group(0, 0, 2, v0)
        conv_group(0, 0, 3, v0)
        v1 = load_image(1)
        conv_img(0, 1, v0)
        v2 = load_image(2)
        conv_img(1, 0, v1)
        conv_img(1, 1, v1)
        v3 = load_image(3)
        conv_img(2, 0, v2)
        conv_img(2, 1, v2)
        conv_img(3, 0, v3)
        conv_img(3, 1, v3, last=True)

    nc.compile()
    return nc


def _get_program():
    if "nc" not in _cached:
        _cached["nc"] = build_program()
    return _cached["nc"]


def kernel(x: np.ndarray, W: np.ndarray, trace: bool = False, **trace_kw):
    nc = _get_program()
    x = np.ascontiguousarray(x, dtype=np.float32)
    # host-side layout permutation only (no arithmetic): [o,i,kh,kw] ->
    # [i,kh,kw,o] so weight taps are contiguous lhsT slices on device
    w_r = np.ascontiguousarray(
        np.asarray(W, dtype=np.float32).transpose(1, 2, 3, 0))
    in_maps = [{"x": x[i * NIMG:(i + 1) * NIMG], "W": w_r}
               for i in range(N_CORES)]
    res = run_bass_kernel_spmd(nc, in_maps, core_ids=list(range(N_CORES)),
                               trace=trace, **trace_kw)
    out = np.concatenate([res.results[i]["y"] for i in range(N_CORES)],
                         axis=0).astype(np.float32)
    if trace:
        return out, res
    return out
